# revision 2
# baseline (speedup 1.0000x reference)
"""ConvDU Trainium2 Bass kernel, v2: fp8 DoubleRowSwInterleave hybrid.

Reference semantics (per batch element, one core per batch element):
    forward  scan t = 1..h-1:   full[t] = relu(conv1x9(full[t-1]) + b) + fea[t]
    backward scan r = h-2..1:   out[r]  = relu(conv1x9(out[r+1]) + b) + full[r]
    out[0] = fea[0], out[h-1] = full[h-1]

Per step the 256->256 1x9 conv is a 2304-deep contraction over (channel, tap)
for each of 2 output-channel tiles. Fast (fp8) steps run it as 18 DoubleRow-
SwInterleave matmuls (contraction 256 = 2 k-tiles per mm, M=128, ~58.6ns
each, 1.9x fp16); slow (fp16) steps as 36 plain matmuls. Precision schedule:
forward scan fp8, backward fp16 (rel err ~1.6e-2 vs the 2e-2 gate; fp8
everywhere measures 2.45e-2).

Key mechanics:
 - relu(z)+row == max(z+row, row): an inject-matmul (stationary s*I,
   rhs = quantized b+row prepped by the Scalar engine) opens each PSUM
   group with s*(b+row), conv matmuls accumulate on top, and the whole
   epilogue per tile is a single DVE op: mir = (ps * 1/s) max row. s=16
   for fp8 steps (weights are scaled 16x into fp8's normal range; the
   activation mirror is unscaled).
 - mirror rows live zero-padded at pitch 144 so every tap is a full-width
   matmul, and tap-pairs map to DoubleRow k-tile dims via manually built
   overlapping-stride APs ([p, {2: stride dk}, {128: stride 1}]).
 - both output tiles share one PSUM bank [128, 2, 128]; the fp32/fp16 row
   store is one combined [*, 2, 128] DVE op; ACT injects per tile.
"""

import numpy as np
import ml_dtypes

E4M3 = ml_dtypes.float8_e4m3

N_CORES = 8
C = 256
H = 128
W = 128
K = 9
PAD = 4
P = 128
CH = 2
PITCH = 144  # mirror row pitch: 4 left pad + 128 + 12 right pad

# tap pairs per output tile: (c2A, kA, c2B, kB)
PAIRS = (
    [(0, 2 * q, 0, 2 * q + 1) for q in range(4)]
    + [(1, 2 * q, 1, 2 * q + 1) for q in range(4)]
    + [(0, 8, 1, 8)]
)

_NC_CACHE = {}


def _build_nc(h, fast_steps, debug_plane=False):
    import concourse.bacc as bacc
    import concourse.mybir as mybir
    import concourse.tile as tile

    nc = bacc.Bacc("TRN2", target_bir_lowering=False, debug=False)
    dt = mybir.dt
    fea_d = nc.dram_tensor("fea", [CH, P, h * W], dt.float32, kind="ExternalInput")
    wsw_d = nc.dram_tensor("wsw", [P, CH * K * 256], dt.float8e4, kind="ExternalInput")
    w16_d = nc.dram_tensor("w16", [P, CH, K, CH, P], dt.float16, kind="ExternalInput")
    bias_d = nc.dram_tensor("bias", [P, 2, CH], dt.float32, kind="ExternalInput")
    id8_d = nc.dram_tensor("id8", [P, P], dt.float16, kind="ExternalInput")
    id16_d = nc.dram_tensor("id16", [P, P], dt.float16, kind="ExternalInput")
    out_d = nc.dram_tensor("out", [CH, P, h * W], dt.float32, kind="ExternalOutput")
    dbg_d = None
    if debug_plane:
        dbg_d = nc.dram_tensor(
            "dbg", [P, h, CH, W], dt.float16, kind="ExternalOutput"
        )

    with tile.TileContext(nc) as tc:
        _convdu(tc, nc, fea_d, wsw_d, w16_d, bias_d, id8_d, id16_d, out_d, h,
                fast_steps, mybir, dbg_d=dbg_d)
    nc.compile()
    return nc


def _convdu(tc, nc, fea_d, wsw_d, w16_d, bias_d, id8_d, id16_d, out_d, h,
            fast_steps, mybir, dbg_d=None):
    from contextlib import ExitStack

    import concourse.bass as bass_mod

    dt = mybir.dt
    f32, f16, f8 = dt.float32, dt.float16, dt.float8e4
    Amax, Amult = mybir.AluOpType.max, mybir.AluOpType.mult
    Ident = mybir.ActivationFunctionType.Identity
    DRSW = mybir.MatmulPerfMode.DoubleRowSwInterleave

    FB = min(32, h)   # fea fp32 rolling rows
    OB = min(32, h)   # out staging rolling rows
    nsteps = 2 * h - 3  # t = 1 .. 2h-3

    def is_fast(t):
        return t in fast_steps

    with ExitStack() as ctx:
        const = ctx.enter_context(tc.tile_pool(name="const", bufs=1))
        psum = ctx.enter_context(tc.tile_pool(name="psum", bufs=2, space="PSUM"))

        plane = const.tile([P, h, CH, W], f16, name="plane")
        fea32 = const.tile([P, FB, CH, W], f32, name="fea32")
        stag = const.tile([P, OB, CH, W], f32, name="stag")
        wsw = const.tile([P, CH * K * 256], f8, name="wsw")
        w16 = const.tile([P, CH, K, CH, P], f16, name="w16")
        bsb = const.tile([P, 2, CH], f32, name="bsb")  # [:, 0, c2]=16b, [:, 1, c2]=b
        mir8 = const.tile([P, 2, CH, PITCH], f8, name="mir8")
        mir16 = const.tile([P, 2, CH, PITCH], f16, name="mir16")
        id16s = const.tile([P, P], f16, name="id16s")  # 16*I fp16
        id16 = const.tile([P, P], f16, name="id16")    # I fp16
        rowb16 = const.tile([P, 2, CH, W], f16, name="rowb16")

        # ---- load order: bias, first fea rows, weights, rest of fea ----
        nc.sync.dma_start(bsb[...], bias_d.ap())
        for j in range(min(3, h)):
            for c2 in range(CH):
                nc.sync.dma_start(
                    fea32[:, j, c2, :], fea_d.ap()[c2, :, j * W : (j + 1) * W]
                )
        nc.sync.dma_start(wsw[...], wsw_d.ap())
        nc.sync.dma_start(w16[...], w16_d.ap())
        nc.sync.dma_start(id16s[...], id8_d.ap())
        nc.sync.dma_start(id16[...], id16_d.ap())
        for j in range(3, FB):
            for c2 in range(CH):
                nc.sync.dma_start(
                    fea32[:, j, c2, :], fea_d.ap()[c2, :, j * W : (j + 1) * W]
                )

        # zero mirror borders once (writes only ever touch [4 : 4+W])
        nc.vector.memset(mir8[...], 0.0)
        nc.vector.memset(mir16[...], 0.0)

        # ---- PE warmup: p-state ramp on dummy matmuls ----
        dummy = const.tile([P, W], f16, name="dummy")
        nc.vector.memset(dummy[:, :], 0.0)
        dps = psum.tile([P, W], f32, tag="wps")
        for _ in range(112):
            nc.tensor.matmul(dps[:, :], dummy[:, :], dummy[:, :], start=True, stop=True)

        # ---- mirror init: slot 0 <- full[0] = fea[0] ----
        if is_fast(1):
            for c2 in range(CH):
                nc.vector.tensor_copy(mir8[:, 0, c2, 4 : 4 + W], fea32[:, 0, c2, :])
        else:
            for c2 in range(CH):
                nc.vector.tensor_copy(mir16[:, 0, c2, 4 : 4 + W], fea32[:, 0, c2, :])

        # ---- AP helpers ----
        wsw_t = wsw[:, :].tensor
        wsw_pstride = CH * K * 256

        def lhsT_sw(o2, q):
            off = (o2 * K + q) * 256
            return bass_mod.AP(
                tensor=wsw_t, ap=[[wsw_pstride, P], [128, 2], [1, 128]], offset=off
            )

        mir8_t = mir8[:, 0, 0, :].tensor
        mir8_pstride = 2 * CH * PITCH

        def rhs_pair(ssrc, q):
            c2a, ka, c2b, kb = PAIRS[q]
            oa = ssrc * CH * PITCH + c2a * PITCH + ka
            ob = ssrc * CH * PITCH + c2b * PITCH + kb
            return bass_mod.AP(
                tensor=mir8_t,
                ap=[[mir8_pstride, P], [ob - oa, 2], [1, W]],
                offset=oa,
            )

        def row_src(t, c2=None):
            # residual row feeding step t's inject and epilogue max
            if t <= h - 1:  # forward: fea row t
                if c2 is None:
                    return fea32[:, t % FB, :, :]
                return fea32[:, t % FB, c2, :]
            r = 2 * h - 2 - t  # backward: full[r]
            if c2 is None:
                return plane[:, r, :, :]
            return plane[:, r, c2, :]

        def prep_rowb(t):
            # rowb = b + row in fp16 (the inject-matmul stationary carries
            # the 16x scale for fp8 steps; fp16 keeps the residual exact-ish)
            for c2 in range(CH):
                nc.scalar.activation(
                    rowb16[:, t % 2, c2, :], row_src(t, c2), Ident,
                    bias=bsb[:, 1, c2 : c2 + 1], scale=1.0,
                )

        def dma_out_rows(r0, r1):
            for c2 in range(CH):
                nc.sync.dma_start(
                    out_d.ap()[c2, :, r0 * W : r1 * W],
                    stag[:, r0 % OB : r0 % OB + (r1 - r0), c2, :],
                )

        # ---- main loop ----
        prep_rowb(1)

        for t in range(1, nsteps + 1):
            fast = is_fast(t)
            ssrc, sdst = (t - 1) % 2, t % 2
            scale_back = 0.0625 if fast else 1.0
            psA = psum.tile([P, W], f32, tag="psA")
            psB = psum.tile([P, W], f32, tag="psB")
            ps2 = (psA, psB)

            if fast:
                # inject-matmuls open the groups, then 18 SW conv mms.
                # Order fills the slot-readiness pipeline: tile0 finishes
                # early so its mirror hides under tile1's matmuls.
                for o2 in range(CH):
                    nc.tensor.matmul(
                        ps2[o2][:, :], id16s[:, :], rowb16[:, t % 2, o2, :],
                        start=True, stop=False,
                    )
                order = (
                    [(0, q) for q in range(4)]
                    + [(1, q) for q in range(2)]
                    + [(0, q) for q in range(4, 9)]
                    + [(1, q) for q in range(2, 9)]
                )
                for o2, q in order:
                    nc.tensor.matmul(
                        ps2[o2][:, :], lhsT_sw(o2, q), rhs_pair(ssrc, q),
                        start=False, stop=(q == 8), perf_mode=DRSW,
                    )
            else:
                for o2 in range(CH):
                    nc.tensor.matmul(
                        ps2[o2][:, :], id16[:, :], rowb16[:, t % 2, o2, :],
                        start=True, stop=False,
                    )
                order = (
                    [(0, 0, k) for k in range(K)]
                    + [(1, 0, k) for k in range(K)]
                    + [(0, 1, k) for k in range(K)]
                    + [(1, 1, k) for k in range(K)]
                )
                for o2, i2, k in order:
                    nc.tensor.matmul(
                        ps2[o2][:, :],
                        w16[:, i2, k, o2, :],
                        mir16[:, ssrc, i2, k : k + W],
                        start=False, stop=(i2 == 1 and k == K - 1),
                    )

            # rowb for the next step (ACT, runs while matmuls stream)
            if t < nsteps:
                prep_rowb(t + 1)

            # epilogue: mirror for step t+1 (critical), one DVE op per tile
            if t < nsteps:
                mdst = mir8 if is_fast(t + 1) else mir16
                for c2 in range(CH):
                    nc.vector.scalar_tensor_tensor(
                        mdst[:, sdst, c2, 4 : 4 + W],
                        ps2[c2][:, :], scale_back, row_src(t, c2),
                        Amult, Amax,
                    )

            # row store (lazy): one DVE op per tile
            if t <= h - 1:
                dsts = [plane[:, t, c2, :] for c2 in range(CH)]
            else:
                r = 2 * h - 2 - t
                dsts = [stag[:, r % OB, c2, :] for c2 in range(CH)]
            for c2 in range(CH):
                nc.vector.scalar_tensor_tensor(
                    dsts[c2], ps2[c2][:, :], scale_back, row_src(t, c2),
                    Amult, Amax,
                )
            if t == h - 1:
                for c2 in range(CH):
                    nc.vector.scalar_tensor_tensor(
                        stag[:, (h - 1) % OB, c2, :], ps2[c2][:, :], scale_back,
                        row_src(t, c2), Amult, Amax,
                    )

            # forward: stream in fea rows; backward: flush output rows
            if t <= h - 1:
                j = t + FB - 3
                if 3 <= FB <= h and FB <= j < h:
                    for c2 in range(CH):
                        nc.sync.dma_start(
                            fea32[:, j % FB, c2, :],
                            fea_d.ap()[c2, :, j * W : (j + 1) * W],
                        )
                if t == h - 1:
                    dma_out_rows(h - 1, h)
            else:
                r = 2 * h - 2 - t
                if r % (OB // 2) == 0 and r > 0:
                    dma_out_rows(r, min(r + OB // 2, h - 1))

        # drain: rows 1..OB//2-1, then row 0 = fea[0] DRAM->DRAM
        if OB // 2 > 1:
            dma_out_rows(1, OB // 2)
        if dbg_d is not None:
            nc.sync.dma_start(dbg_d.ap(), plane[...])
        for c2 in range(CH):
            nc.sync.dma_start(out_d.ap()[c2, :, 0:W], fea_d.ap()[c2, :, 0:W])


def _prep_static(weight, bias, h):
    """Host-side packing: SwInterleave fp8 weights (x16), fp16 weights, biases."""
    w = np.asarray(weight, np.float32).reshape(CH, P, CH, P, K)  # [o2,o,i2,i,k]
    # fp16 (unscaled): w16[i(part), i2, k, o2, m] = w[o2, m, i2, i, k]
    w16 = np.ascontiguousarray(w.transpose(3, 2, 4, 0, 1)).astype(np.float16)
    # fp8 SwInterleave (x16): per (o2, pair q): ktile mats A,B [i(part), m]
    w8 = (w * 16.0).astype(E4M3).astype(np.float32)
    wsw = np.zeros((P, CH, K, 256), np.float32)
    for q, (c2a, ka, c2b, kb) in enumerate(PAIRS):
        for o2 in range(CH):
            A = w8[o2, :, c2a, :, ka].T  # [i(part), m(out)]
            B = w8[o2, :, c2b, :, kb].T
            wsw[:, o2, q, :] = np.stack(
                [A[:, ::-1], B[:, ::-1]], axis=2
            ).reshape(P, 256)
    wsw = np.ascontiguousarray(wsw.reshape(P, CH * K * 256)).astype(E4M3)
    b = np.asarray(bias, np.float32).reshape(CH, P)
    bsb = np.stack([16.0 * b.T, b.T], axis=1)  # [P, 2, CH]
    bsb = np.ascontiguousarray(bsb).astype(np.float32)
    id8 = (np.eye(P) * 16.0).astype(np.float16)
    id16 = np.eye(P, dtype=np.float16)
    return wsw, w16, bsb, id8, id16


def default_fast_steps(h):
    # forward scan fp8; 32 backward steps (spread evenly) also fp8.
    # Simulated rel err 1.86e-2 vs the 2e-2 gate (fwd-only: 1.62e-2).
    fs = set(range(1, h))
    bwd = list(range(h, 2 * h - 2))
    if len(bwd) > 40:
        fs |= set(bwd[:32])
    return frozenset(fs)


def run(fea, weight, bias, trace=False, fast_steps=None, **spmd_kwargs):
    from concourse.bass_utils import run_bass_kernel_spmd

    fea = np.asarray(fea, dtype=np.float32)
    n, c, h, w = fea.shape
    assert c == C and w == W
    if fast_steps is None:
        fast_steps = default_fast_steps(h)
    fast_steps = frozenset(fast_steps)
    wsw, w16, bsb, id8, id16 = _prep_static(weight, bias, h)
    in_maps = []
    for bi in range(n):
        feab = np.ascontiguousarray(fea[bi].reshape(CH, P, h * W))
        in_maps.append({"fea": feab, "wsw": wsw, "w16": w16, "bias": bsb,
                        "id8": id8, "id16": id16})
    key = (h, fast_steps, bool(spmd_kwargs.pop("debug_plane", False)))
    if key not in _NC_CACHE:
        _NC_CACHE[key] = _build_nc(h, fast_steps, debug_plane=key[2])
    nc = _NC_CACHE[key]
    try:
        res = run_bass_kernel_spmd(
            nc, in_maps, core_ids=list(range(n)), trace=trace, **spmd_kwargs
        )
    except Exception:
        res = run_bass_kernel_spmd(
            nc, in_maps, core_ids=list(range(n)), trace=trace, **spmd_kwargs
        )
    outs = [res.results[bi]["out"].reshape(C, h, W) for bi in range(n)]
    return np.stack(outs, axis=0).astype(np.float32), res


def kernel(fea, weight, bias):
    out, _ = run(fea, weight, bias, trace=False)
    return out


# revision 3
# speedup vs baseline: 1.0081x; 1.0081x over previous
"""ConvDU Trainium2 Bass kernel, v2: fp8 DoubleRowSwInterleave hybrid.

Reference semantics (per batch element, one core per batch element):
    forward  scan t = 1..h-1:   full[t] = relu(conv1x9(full[t-1]) + b) + fea[t]
    backward scan r = h-2..1:   out[r]  = relu(conv1x9(out[r+1]) + b) + full[r]
    out[0] = fea[0], out[h-1] = full[h-1]

Per step the 256->256 1x9 conv is a 2304-deep contraction over (channel, tap)
for each of 2 output-channel tiles. Fast (fp8) steps run it as 18 DoubleRow-
SwInterleave matmuls (contraction 256 = 2 k-tiles per mm, M=128, ~58.6ns
each, 1.9x fp16); slow (fp16) steps as 36 plain matmuls. Precision schedule:
forward scan fp8, backward fp16 (rel err ~1.6e-2 vs the 2e-2 gate; fp8
everywhere measures 2.45e-2).

Key mechanics:
 - relu(z)+row == max(z+row, row): an inject-matmul (stationary s*I,
   rhs = quantized b+row prepped by the Scalar engine) opens each PSUM
   group with s*(b+row), conv matmuls accumulate on top, and the whole
   epilogue per tile is a single DVE op: mir = (ps * 1/s) max row. s=16
   for fp8 steps (weights are scaled 16x into fp8's normal range; the
   activation mirror is unscaled).
 - mirror rows live zero-padded at pitch 144 so every tap is a full-width
   matmul, and tap-pairs map to DoubleRow k-tile dims via manually built
   overlapping-stride APs ([p, {2: stride dk}, {128: stride 1}]).
 - both output tiles share one PSUM bank [128, 2, 128]; the fp32/fp16 row
   store is one combined [*, 2, 128] DVE op; ACT injects per tile.
"""

import numpy as np
import ml_dtypes

E4M3 = ml_dtypes.float8_e4m3

N_CORES = 8
C = 256
H = 128
W = 128
K = 9
PAD = 4
P = 128
CH = 2
PITCH = 144  # mirror row pitch: 4 left pad + 128 + 12 right pad

# tap pairs per output tile: (c2A, kA, c2B, kB)
PAIRS = (
    [(0, 2 * q, 0, 2 * q + 1) for q in range(4)]
    + [(1, 2 * q, 1, 2 * q + 1) for q in range(4)]
    + [(0, 8, 1, 8)]
)

_NC_CACHE = {}


def _build_nc(h, fast_steps, debug_plane=False):
    import concourse.bacc as bacc
    import concourse.mybir as mybir
    import concourse.tile as tile

    nc = bacc.Bacc("TRN2", target_bir_lowering=False, debug=False)
    dt = mybir.dt
    fea_d = nc.dram_tensor("fea", [CH, P, h * W], dt.float32, kind="ExternalInput")
    wsw_d = nc.dram_tensor("wsw", [P, CH * K * 256], dt.float8e4, kind="ExternalInput")
    w16_d = nc.dram_tensor("w16", [P, CH, K, CH, P], dt.float16, kind="ExternalInput")
    bias_d = nc.dram_tensor("bias", [P, 2, CH], dt.float32, kind="ExternalInput")
    id8_d = nc.dram_tensor("id8", [P, P], dt.float16, kind="ExternalInput")
    id16_d = nc.dram_tensor("id16", [P, P], dt.float16, kind="ExternalInput")
    out_d = nc.dram_tensor("out", [CH, P, h * W], dt.float32, kind="ExternalOutput")
    dbg_d = None
    if debug_plane:
        dbg_d = nc.dram_tensor(
            "dbg", [P, h, CH, W], dt.float16, kind="ExternalOutput"
        )

    with tile.TileContext(nc) as tc:
        _convdu(tc, nc, fea_d, wsw_d, w16_d, bias_d, id8_d, id16_d, out_d, h,
                fast_steps, mybir, dbg_d=dbg_d)
    nc.compile()
    return nc


def _convdu(tc, nc, fea_d, wsw_d, w16_d, bias_d, id8_d, id16_d, out_d, h,
            fast_steps, mybir, dbg_d=None):
    from contextlib import ExitStack

    import concourse.bass as bass_mod

    dt = mybir.dt
    f32, f16, f8 = dt.float32, dt.float16, dt.float8e4
    Amax, Amult = mybir.AluOpType.max, mybir.AluOpType.mult
    Ident = mybir.ActivationFunctionType.Identity
    DRSW = mybir.MatmulPerfMode.DoubleRowSwInterleave

    FB = min(32, h)   # fea fp32 rolling rows
    OB = min(32, h)   # out staging rolling rows
    nsteps = 2 * h - 3  # t = 1 .. 2h-3

    def is_fast(t):
        return t in fast_steps

    with ExitStack() as ctx:
        const = ctx.enter_context(tc.tile_pool(name="const", bufs=1))
        psum = ctx.enter_context(tc.tile_pool(name="psum", bufs=2, space="PSUM"))

        plane = const.tile([P, h, CH, W], f16, name="plane")
        fea32 = const.tile([P, FB, CH, W], f32, name="fea32")
        stag = const.tile([P, OB, CH, W], f32, name="stag")
        wsw = const.tile([P, CH * K * 256], f8, name="wsw")
        w16 = const.tile([P, CH, K, CH, P], f16, name="w16")
        bsb = const.tile([P, 2, CH], f32, name="bsb")  # [:, 0, c2]=16b, [:, 1, c2]=b
        mir8 = const.tile([P, 2, CH, PITCH], f8, name="mir8")
        mir16 = const.tile([P, 2, CH, PITCH], f16, name="mir16")
        id16s = const.tile([P, P], f16, name="id16s")  # 16*I fp16
        id16 = const.tile([P, P], f16, name="id16")    # I fp16
        rowb16 = const.tile([P, 2, CH, W], f16, name="rowb16")

        # ---- load order: bias, first fea rows, weights, rest of fea ----
        nc.sync.dma_start(bsb[...], bias_d.ap())
        for j in range(min(3, h)):
            for c2 in range(CH):
                nc.sync.dma_start(
                    fea32[:, j, c2, :], fea_d.ap()[c2, :, j * W : (j + 1) * W]
                )
        nc.sync.dma_start(wsw[...], wsw_d.ap())
        nc.sync.dma_start(w16[...], w16_d.ap())
        nc.sync.dma_start(id16s[...], id8_d.ap())
        nc.sync.dma_start(id16[...], id16_d.ap())
        for j in range(3, FB):
            for c2 in range(CH):
                nc.sync.dma_start(
                    fea32[:, j, c2, :], fea_d.ap()[c2, :, j * W : (j + 1) * W]
                )

        # zero mirror borders once (writes only ever touch [4 : 4+W])
        nc.vector.memset(mir8[...], 0.0)
        nc.vector.memset(mir16[...], 0.0)

        # ---- PE warmup: p-state ramp on dummy matmuls ----
        dummy = const.tile([P, W], f16, name="dummy")
        nc.vector.memset(dummy[:, :], 0.0)
        dps = psum.tile([P, W], f32, tag="wps")
        for _ in range(112):
            nc.tensor.matmul(dps[:, :], dummy[:, :], dummy[:, :], start=True, stop=True)

        # ---- mirror init: slot 0 <- full[0] = fea[0] ----
        if is_fast(1):
            for c2 in range(CH):
                nc.vector.tensor_copy(mir8[:, 0, c2, 4 : 4 + W], fea32[:, 0, c2, :])
        else:
            for c2 in range(CH):
                nc.vector.tensor_copy(mir16[:, 0, c2, 4 : 4 + W], fea32[:, 0, c2, :])

        # ---- AP helpers ----
        wsw_t = wsw[:, :].tensor
        wsw_pstride = CH * K * 256

        def lhsT_sw(o2, q):
            off = (o2 * K + q) * 256
            return bass_mod.AP(
                tensor=wsw_t, ap=[[wsw_pstride, P], [128, 2], [1, 128]], offset=off
            )

        mir8_t = mir8[:, 0, 0, :].tensor
        mir8_pstride = 2 * CH * PITCH

        def rhs_pair(ssrc, q):
            c2a, ka, c2b, kb = PAIRS[q]
            oa = ssrc * CH * PITCH + c2a * PITCH + ka
            ob = ssrc * CH * PITCH + c2b * PITCH + kb
            return bass_mod.AP(
                tensor=mir8_t,
                ap=[[mir8_pstride, P], [ob - oa, 2], [1, W]],
                offset=oa,
            )

        def row_src(t, c2=None):
            # residual row feeding step t's inject and epilogue max
            if t <= h - 1:  # forward: fea row t
                if c2 is None:
                    return fea32[:, t % FB, :, :]
                return fea32[:, t % FB, c2, :]
            r = 2 * h - 2 - t  # backward: full[r]
            if c2 is None:
                return plane[:, r, :, :]
            return plane[:, r, c2, :]

        def prep_rowb(t):
            # rowb = b + row in fp16 (the inject-matmul stationary carries
            # the 16x scale for fp8 steps; fp16 keeps the residual exact-ish)
            for c2 in range(CH):
                nc.scalar.activation(
                    rowb16[:, t % 2, c2, :], row_src(t, c2), Ident,
                    bias=bsb[:, 1, c2 : c2 + 1], scale=1.0,
                )

        def dma_out_rows(r0, r1):
            for c2 in range(CH):
                nc.sync.dma_start(
                    out_d.ap()[c2, :, r0 * W : r1 * W],
                    stag[:, r0 % OB : r0 % OB + (r1 - r0), c2, :],
                )

        # ---- main loop ----
        prep_rowb(1)

        for t in range(1, nsteps + 1):
            fast = is_fast(t)
            ssrc, sdst = (t - 1) % 2, t % 2
            scale_back = 0.0625 if fast else 1.0
            psA = psum.tile([P, W], f32, tag="psA")
            psB = psum.tile([P, W], f32, tag="psB")
            ps2 = (psA, psB)

            if fast:
                # inject-matmuls open the groups, then 18 SW conv mms.
                # Order fills the slot-readiness pipeline: tile0 finishes
                # early so its mirror hides under tile1's matmuls.
                for o2 in range(CH):
                    nc.tensor.matmul(
                        ps2[o2][:, :], id16s[:, :], rowb16[:, t % 2, o2, :],
                        start=True, stop=False,
                    )
                order = (
                    [(0, q) for q in range(4)]
                    + [(1, q) for q in range(2)]
                    + [(0, q) for q in range(4, 9)]
                    + [(1, q) for q in range(2, 9)]
                )
                for o2, q in order:
                    nc.tensor.matmul(
                        ps2[o2][:, :], lhsT_sw(o2, q), rhs_pair(ssrc, q),
                        start=False, stop=(q == 8), perf_mode=DRSW,
                    )
            else:
                for o2 in range(CH):
                    nc.tensor.matmul(
                        ps2[o2][:, :], id16[:, :], rowb16[:, t % 2, o2, :],
                        start=True, stop=False,
                    )
                order = (
                    [(0, 0, k) for k in range(K)]
                    + [(1, 0, k) for k in range(K)]
                    + [(0, 1, k) for k in range(K)]
                    + [(1, 1, k) for k in range(K)]
                )
                for o2, i2, k in order:
                    nc.tensor.matmul(
                        ps2[o2][:, :],
                        w16[:, i2, k, o2, :],
                        mir16[:, ssrc, i2, k : k + W],
                        start=False, stop=(i2 == 1 and k == K - 1),
                    )

            # rowb for the next step (ACT, runs while matmuls stream)
            if t < nsteps:
                prep_rowb(t + 1)

            # epilogue: mirror for step t+1 (critical), one DVE op per tile
            if t < nsteps:
                mdst = mir8 if is_fast(t + 1) else mir16
                for c2 in range(CH):
                    nc.vector.scalar_tensor_tensor(
                        mdst[:, sdst, c2, 4 : 4 + W],
                        ps2[c2][:, :], scale_back, row_src(t, c2),
                        Amult, Amax,
                    )

            # row store (lazy): one DVE op per tile
            if t <= h - 1:
                dsts = [plane[:, t, c2, :] for c2 in range(CH)]
            else:
                r = 2 * h - 2 - t
                dsts = [stag[:, r % OB, c2, :] for c2 in range(CH)]
            for c2 in range(CH):
                nc.vector.scalar_tensor_tensor(
                    dsts[c2], ps2[c2][:, :], scale_back, row_src(t, c2),
                    Amult, Amax,
                )
            if t == h - 1:
                for c2 in range(CH):
                    nc.vector.scalar_tensor_tensor(
                        stag[:, (h - 1) % OB, c2, :], ps2[c2][:, :], scale_back,
                        row_src(t, c2), Amult, Amax,
                    )

            # forward: stream in fea rows; backward: flush output rows
            if t <= h - 1:
                j = t + FB - 3
                if 3 <= FB <= h and FB <= j < h:
                    for c2 in range(CH):
                        nc.sync.dma_start(
                            fea32[:, j % FB, c2, :],
                            fea_d.ap()[c2, :, j * W : (j + 1) * W],
                        )
                if t == h - 1:
                    dma_out_rows(h - 1, h)
            else:
                r = 2 * h - 2 - t
                if r % (OB // 2) == 0 and r > 0:
                    dma_out_rows(r, min(r + OB // 2, h - 1))

        # drain: rows 1..OB//2-1, then row 0 = fea[0] DRAM->DRAM
        if OB // 2 > 1:
            dma_out_rows(1, OB // 2)
        if dbg_d is not None:
            nc.sync.dma_start(dbg_d.ap(), plane[...])
        for c2 in range(CH):
            nc.sync.dma_start(out_d.ap()[c2, :, 0:W], fea_d.ap()[c2, :, 0:W])


def _prep_static(weight, bias, h):
    """Host-side packing: SwInterleave fp8 weights (x16), fp16 weights, biases."""
    w = np.asarray(weight, np.float32).reshape(CH, P, CH, P, K)  # [o2,o,i2,i,k]
    # fp16 (unscaled): w16[i(part), i2, k, o2, m] = w[o2, m, i2, i, k]
    w16 = np.ascontiguousarray(w.transpose(3, 2, 4, 0, 1)).astype(np.float16)
    # fp8 SwInterleave (x16): per (o2, pair q): ktile mats A,B [i(part), m]
    w8 = (w * 16.0).astype(E4M3).astype(np.float32)
    wsw = np.zeros((P, CH, K, 256), np.float32)
    for q, (c2a, ka, c2b, kb) in enumerate(PAIRS):
        for o2 in range(CH):
            A = w8[o2, :, c2a, :, ka].T  # [i(part), m(out)]
            B = w8[o2, :, c2b, :, kb].T
            wsw[:, o2, q, :] = np.stack(
                [A[:, ::-1], B[:, ::-1]], axis=2
            ).reshape(P, 256)
    wsw = np.ascontiguousarray(wsw.reshape(P, CH * K * 256)).astype(E4M3)
    b = np.asarray(bias, np.float32).reshape(CH, P)
    bsb = np.stack([16.0 * b.T, b.T], axis=1)  # [P, 2, CH]
    bsb = np.ascontiguousarray(bsb).astype(np.float32)
    id8 = (np.eye(P) * 16.0).astype(np.float16)
    id16 = np.eye(P, dtype=np.float16)
    return wsw, w16, bsb, id8, id16


def default_fast_steps(h):
    # forward scan fp8; 32 backward steps (spread evenly) also fp8.
    # Simulated rel err 1.86e-2 vs the 2e-2 gate (fwd-only: 1.62e-2).
    fs = set(range(1, h))
    bwd = list(range(h, 2 * h - 2))
    if len(bwd) > 40:
        fs |= set(bwd[:40])
    return frozenset(fs)


def run(fea, weight, bias, trace=False, fast_steps=None, **spmd_kwargs):
    from concourse.bass_utils import run_bass_kernel_spmd

    fea = np.asarray(fea, dtype=np.float32)
    n, c, h, w = fea.shape
    assert c == C and w == W
    if fast_steps is None:
        fast_steps = default_fast_steps(h)
    fast_steps = frozenset(fast_steps)
    wsw, w16, bsb, id8, id16 = _prep_static(weight, bias, h)
    in_maps = []
    for bi in range(n):
        feab = np.ascontiguousarray(fea[bi].reshape(CH, P, h * W))
        in_maps.append({"fea": feab, "wsw": wsw, "w16": w16, "bias": bsb,
                        "id8": id8, "id16": id16})
    key = (h, fast_steps, bool(spmd_kwargs.pop("debug_plane", False)))
    if key not in _NC_CACHE:
        _NC_CACHE[key] = _build_nc(h, fast_steps, debug_plane=key[2])
    nc = _NC_CACHE[key]
    try:
        res = run_bass_kernel_spmd(
            nc, in_maps, core_ids=list(range(n)), trace=trace, **spmd_kwargs
        )
    except Exception:
        res = run_bass_kernel_spmd(
            nc, in_maps, core_ids=list(range(n)), trace=trace, **spmd_kwargs
        )
    outs = [res.results[bi]["out"].reshape(C, h, W) for bi in range(n)]
    return np.stack(outs, axis=0).astype(np.float32), res


def kernel(fea, weight, bias):
    out, _ = run(fea, weight, bias, trace=False)
    return out


# revision 4
# speedup vs baseline: 1.0106x; 1.0025x over previous
"""ConvDU Trainium2 Bass kernel, v2: fp8 DoubleRowSwInterleave hybrid.

Reference semantics (per batch element, one core per batch element):
    forward  scan t = 1..h-1:   full[t] = relu(conv1x9(full[t-1]) + b) + fea[t]
    backward scan r = h-2..1:   out[r]  = relu(conv1x9(out[r+1]) + b) + full[r]
    out[0] = fea[0], out[h-1] = full[h-1]

Per step the 256->256 1x9 conv is a 2304-deep contraction over (channel, tap)
for each of 2 output-channel tiles. Fast (fp8) steps run it as 18 DoubleRow-
SwInterleave matmuls (contraction 256 = 2 k-tiles per mm, M=128, ~58.6ns
each, 1.9x fp16); slow (fp16) steps as 36 plain matmuls. Precision schedule:
forward scan fp8, backward fp16 (rel err ~1.6e-2 vs the 2e-2 gate; fp8
everywhere measures 2.45e-2).

Key mechanics:
 - relu(z)+row == max(z+row, row): an inject-matmul (stationary s*I,
   rhs = quantized b+row prepped by the Scalar engine) opens each PSUM
   group with s*(b+row), conv matmuls accumulate on top, and the whole
   epilogue per tile is a single DVE op: mir = (ps * 1/s) max row. s=16
   for fp8 steps (weights are scaled 16x into fp8's normal range; the
   activation mirror is unscaled).
 - mirror rows live zero-padded at pitch 144 so every tap is a full-width
   matmul, and tap-pairs map to DoubleRow k-tile dims via manually built
   overlapping-stride APs ([p, {2: stride dk}, {128: stride 1}]).
 - both output tiles share one PSUM bank [128, 2, 128]; the fp32/fp16 row
   store is one combined [*, 2, 128] DVE op; ACT injects per tile.
"""

import numpy as np
import ml_dtypes

E4M3 = ml_dtypes.float8_e4m3

N_CORES = 8
C = 256
H = 128
W = 128
K = 9
PAD = 4
P = 128
CH = 2
PITCH = 144  # mirror row pitch: 4 left pad + 128 + 12 right pad

# tap pairs per output tile: (c2A, kA, c2B, kB)
PAIRS = (
    [(0, 2 * q, 0, 2 * q + 1) for q in range(4)]
    + [(1, 2 * q, 1, 2 * q + 1) for q in range(4)]
    + [(0, 8, 1, 8)]
)

_NC_CACHE = {}


def _build_nc(h, fast_steps, debug_plane=False):
    import concourse.bacc as bacc
    import concourse.mybir as mybir
    import concourse.tile as tile

    nc = bacc.Bacc("TRN2", target_bir_lowering=False, debug=False)
    dt = mybir.dt
    fea_d = nc.dram_tensor("fea", [CH, P, h * W], dt.float32, kind="ExternalInput")
    wsw_d = nc.dram_tensor("wsw", [P, CH * K * 256], dt.float8e4, kind="ExternalInput")
    w16_d = nc.dram_tensor("w16", [P, CH, K, CH, P], dt.float16, kind="ExternalInput")
    bias_d = nc.dram_tensor("bias", [P, 2, CH], dt.float32, kind="ExternalInput")
    id8_d = nc.dram_tensor("id8", [P, P], dt.float16, kind="ExternalInput")
    id16_d = nc.dram_tensor("id16", [P, P], dt.float16, kind="ExternalInput")
    out_d = nc.dram_tensor("out", [CH, P, h * W], dt.float32, kind="ExternalOutput")
    dbg_d = None
    if debug_plane:
        dbg_d = nc.dram_tensor(
            "dbg", [P, h, CH, W], dt.float16, kind="ExternalOutput"
        )

    with tile.TileContext(nc) as tc:
        _convdu(tc, nc, fea_d, wsw_d, w16_d, bias_d, id8_d, id16_d, out_d, h,
                fast_steps, mybir, dbg_d=dbg_d)
    nc.compile()
    return nc


def _convdu(tc, nc, fea_d, wsw_d, w16_d, bias_d, id8_d, id16_d, out_d, h,
            fast_steps, mybir, dbg_d=None):
    from contextlib import ExitStack

    import concourse.bass as bass_mod

    dt = mybir.dt
    f32, f16, f8 = dt.float32, dt.float16, dt.float8e4
    Amax, Amult = mybir.AluOpType.max, mybir.AluOpType.mult
    Ident = mybir.ActivationFunctionType.Identity
    DRSW = mybir.MatmulPerfMode.DoubleRowSwInterleave

    FB = min(32, h)   # fea fp32 rolling rows
    OB = min(32, h)   # out staging rolling rows
    nsteps = 2 * h - 3  # t = 1 .. 2h-3

    def is_fast(t):
        return t in fast_steps

    with ExitStack() as ctx:
        const = ctx.enter_context(tc.tile_pool(name="const", bufs=1))
        psum = ctx.enter_context(tc.tile_pool(name="psum", bufs=2, space="PSUM"))

        plane = const.tile([P, h, CH, W], f16, name="plane")
        fea32 = const.tile([P, FB, CH, W], f32, name="fea32")
        stag = const.tile([P, OB, CH, W], f32, name="stag")
        wsw = const.tile([P, CH * K * 256], f8, name="wsw")
        w16 = const.tile([P, CH, K, CH, P], f16, name="w16")
        bsb = const.tile([P, 2, CH], f32, name="bsb")  # [:, 0, c2]=16b, [:, 1, c2]=b
        mir8 = const.tile([P, 2, CH, PITCH], f8, name="mir8")
        mir16 = const.tile([P, 2, CH, PITCH], f16, name="mir16")
        id16s = const.tile([P, P], f16, name="id16s")  # 16*I fp16
        id16 = const.tile([P, P], f16, name="id16")    # I fp16
        rowb16 = const.tile([P, 2, CH, W], f16, name="rowb16")

        # ---- load order: bias, first fea rows, weights, rest of fea ----
        nc.sync.dma_start(bsb[...], bias_d.ap())
        for j in range(min(3, h)):
            for c2 in range(CH):
                nc.sync.dma_start(
                    fea32[:, j, c2, :], fea_d.ap()[c2, :, j * W : (j + 1) * W]
                )
        nc.sync.dma_start(wsw[...], wsw_d.ap())
        nc.sync.dma_start(w16[...], w16_d.ap())
        nc.sync.dma_start(id16s[...], id8_d.ap())
        nc.sync.dma_start(id16[...], id16_d.ap())
        for j in range(3, FB):
            for c2 in range(CH):
                nc.sync.dma_start(
                    fea32[:, j, c2, :], fea_d.ap()[c2, :, j * W : (j + 1) * W]
                )

        # zero mirror borders once (writes only ever touch [4 : 4+W])
        nc.vector.memset(mir8[...], 0.0)
        nc.vector.memset(mir16[...], 0.0)

        # ---- PE warmup: p-state ramp on dummy matmuls ----
        dummy = const.tile([P, W], f16, name="dummy")
        nc.vector.memset(dummy[:, :], 0.0)
        dps = psum.tile([P, W], f32, tag="wps")
        for _ in range(64):
            nc.tensor.matmul(dps[:, :], dummy[:, :], dummy[:, :], start=True, stop=True)

        # ---- mirror init: slot 0 <- full[0] = fea[0] ----
        if is_fast(1):
            for c2 in range(CH):
                nc.vector.tensor_copy(mir8[:, 0, c2, 4 : 4 + W], fea32[:, 0, c2, :])
        else:
            for c2 in range(CH):
                nc.vector.tensor_copy(mir16[:, 0, c2, 4 : 4 + W], fea32[:, 0, c2, :])

        # ---- AP helpers ----
        wsw_t = wsw[:, :].tensor
        wsw_pstride = CH * K * 256

        def lhsT_sw(o2, q):
            off = (o2 * K + q) * 256
            return bass_mod.AP(
                tensor=wsw_t, ap=[[wsw_pstride, P], [128, 2], [1, 128]], offset=off
            )

        mir8_t = mir8[:, 0, 0, :].tensor
        mir8_pstride = 2 * CH * PITCH

        def rhs_pair(ssrc, q):
            c2a, ka, c2b, kb = PAIRS[q]
            oa = ssrc * CH * PITCH + c2a * PITCH + ka
            ob = ssrc * CH * PITCH + c2b * PITCH + kb
            return bass_mod.AP(
                tensor=mir8_t,
                ap=[[mir8_pstride, P], [ob - oa, 2], [1, W]],
                offset=oa,
            )

        def row_src(t, c2=None):
            # residual row feeding step t's inject and epilogue max
            if t <= h - 1:  # forward: fea row t
                if c2 is None:
                    return fea32[:, t % FB, :, :]
                return fea32[:, t % FB, c2, :]
            r = 2 * h - 2 - t  # backward: full[r]
            if c2 is None:
                return plane[:, r, :, :]
            return plane[:, r, c2, :]

        def prep_rowb(t):
            # rowb = b + row in fp16 (the inject-matmul stationary carries
            # the 16x scale for fp8 steps; fp16 keeps the residual exact-ish)
            for c2 in range(CH):
                nc.scalar.activation(
                    rowb16[:, t % 2, c2, :], row_src(t, c2), Ident,
                    bias=bsb[:, 1, c2 : c2 + 1], scale=1.0,
                )

        def dma_out_rows(r0, r1):
            for c2 in range(CH):
                nc.sync.dma_start(
                    out_d.ap()[c2, :, r0 * W : r1 * W],
                    stag[:, r0 % OB : r0 % OB + (r1 - r0), c2, :],
                )

        # ---- main loop ----
        prep_rowb(1)

        for t in range(1, nsteps + 1):
            fast = is_fast(t)
            ssrc, sdst = (t - 1) % 2, t % 2
            scale_back = 0.0625 if fast else 1.0
            psA = psum.tile([P, W], f32, tag="psA")
            psB = psum.tile([P, W], f32, tag="psB")
            ps2 = (psA, psB)

            if fast:
                # inject-matmuls open the groups, then 18 SW conv mms.
                # Order fills the slot-readiness pipeline: tile0 finishes
                # early so its mirror hides under tile1's matmuls.
                for o2 in range(CH):
                    nc.tensor.matmul(
                        ps2[o2][:, :], id16s[:, :], rowb16[:, t % 2, o2, :],
                        start=True, stop=False,
                    )
                order = (
                    [(0, q) for q in range(4)]
                    + [(1, q) for q in range(2)]
                    + [(0, q) for q in range(4, 9)]
                    + [(1, q) for q in range(2, 9)]
                )
                for o2, q in order:
                    nc.tensor.matmul(
                        ps2[o2][:, :], lhsT_sw(o2, q), rhs_pair(ssrc, q),
                        start=False, stop=(q == 8), perf_mode=DRSW,
                    )
            else:
                for o2 in range(CH):
                    nc.tensor.matmul(
                        ps2[o2][:, :], id16[:, :], rowb16[:, t % 2, o2, :],
                        start=True, stop=False,
                    )
                order = (
                    [(0, 0, k) for k in range(K)]
                    + [(1, 0, k) for k in range(K)]
                    + [(0, 1, k) for k in range(K)]
                    + [(1, 1, k) for k in range(K)]
                )
                for o2, i2, k in order:
                    nc.tensor.matmul(
                        ps2[o2][:, :],
                        w16[:, i2, k, o2, :],
                        mir16[:, ssrc, i2, k : k + W],
                        start=False, stop=(i2 == 1 and k == K - 1),
                    )

            # rowb for the next step (ACT, runs while matmuls stream)
            if t < nsteps:
                prep_rowb(t + 1)

            # epilogue, interleaved per tile as [mir0, plane0, mir1, plane1]
            # so the non-critical plane0 (psA-only) doesn't queue behind mir1
            # (which must wait for step-end psB) - keeps the in-order DVE
            # draining early; it is saturated during fast steps.
            if t <= h - 1:
                dsts = [plane[:, t, c2, :] for c2 in range(CH)]
            else:
                r = 2 * h - 2 - t
                dsts = [stag[:, r % OB, c2, :] for c2 in range(CH)]
            mdst = mir8 if is_fast(t + 1) else mir16
            for c2 in range(CH):
                if t < nsteps:
                    nc.vector.scalar_tensor_tensor(
                        mdst[:, sdst, c2, 4 : 4 + W],
                        ps2[c2][:, :], scale_back, row_src(t, c2),
                        Amult, Amax,
                    )
                nc.vector.scalar_tensor_tensor(
                    dsts[c2], ps2[c2][:, :], scale_back, row_src(t, c2),
                    Amult, Amax,
                )
            if t == h - 1:
                for c2 in range(CH):
                    nc.vector.scalar_tensor_tensor(
                        stag[:, (h - 1) % OB, c2, :], ps2[c2][:, :], scale_back,
                        row_src(t, c2), Amult, Amax,
                    )

            # forward: stream in fea rows; backward: flush output rows
            if t <= h - 1:
                j = t + FB - 3
                if 3 <= FB <= h and FB <= j < h:
                    for c2 in range(CH):
                        nc.sync.dma_start(
                            fea32[:, j % FB, c2, :],
                            fea_d.ap()[c2, :, j * W : (j + 1) * W],
                        )
                if t == h - 1:
                    dma_out_rows(h - 1, h)
            else:
                r = 2 * h - 2 - t
                if r % (OB // 2) == 0 and r > 0:
                    dma_out_rows(r, min(r + OB // 2, h - 1))

        # drain: rows 1..OB//2-1, then row 0 = fea[0] DRAM->DRAM
        if OB // 2 > 1:
            dma_out_rows(1, OB // 2)
        if dbg_d is not None:
            nc.sync.dma_start(dbg_d.ap(), plane[...])
        for c2 in range(CH):
            nc.sync.dma_start(out_d.ap()[c2, :, 0:W], fea_d.ap()[c2, :, 0:W])


def _prep_static(weight, bias, h):
    """Host-side packing: SwInterleave fp8 weights (x16), fp16 weights, biases."""
    w = np.asarray(weight, np.float32).reshape(CH, P, CH, P, K)  # [o2,o,i2,i,k]
    # fp16 (unscaled): w16[i(part), i2, k, o2, m] = w[o2, m, i2, i, k]
    w16 = np.ascontiguousarray(w.transpose(3, 2, 4, 0, 1)).astype(np.float16)
    # fp8 SwInterleave (x16): per (o2, pair q): ktile mats A,B [i(part), m]
    w8 = (w * 16.0).astype(E4M3).astype(np.float32)
    wsw = np.zeros((P, CH, K, 256), np.float32)
    for q, (c2a, ka, c2b, kb) in enumerate(PAIRS):
        for o2 in range(CH):
            A = w8[o2, :, c2a, :, ka].T  # [i(part), m(out)]
            B = w8[o2, :, c2b, :, kb].T
            wsw[:, o2, q, :] = np.stack(
                [A[:, ::-1], B[:, ::-1]], axis=2
            ).reshape(P, 256)
    wsw = np.ascontiguousarray(wsw.reshape(P, CH * K * 256)).astype(E4M3)
    b = np.asarray(bias, np.float32).reshape(CH, P)
    bsb = np.stack([16.0 * b.T, b.T], axis=1)  # [P, 2, CH]
    bsb = np.ascontiguousarray(bsb).astype(np.float32)
    id8 = (np.eye(P) * 16.0).astype(np.float16)
    id16 = np.eye(P, dtype=np.float16)
    return wsw, w16, bsb, id8, id16


def default_fast_steps(h):
    # forward scan fp8; 32 backward steps (spread evenly) also fp8.
    # Simulated rel err 1.86e-2 vs the 2e-2 gate (fwd-only: 1.62e-2).
    fs = set(range(1, h))
    bwd = list(range(h, 2 * h - 2))
    if len(bwd) > 40:
        fs |= set(bwd[:40])
    return frozenset(fs)


def run(fea, weight, bias, trace=False, fast_steps=None, **spmd_kwargs):
    from concourse.bass_utils import run_bass_kernel_spmd

    fea = np.asarray(fea, dtype=np.float32)
    n, c, h, w = fea.shape
    assert c == C and w == W
    if fast_steps is None:
        fast_steps = default_fast_steps(h)
    fast_steps = frozenset(fast_steps)
    wsw, w16, bsb, id8, id16 = _prep_static(weight, bias, h)
    in_maps = []
    for bi in range(n):
        feab = np.ascontiguousarray(fea[bi].reshape(CH, P, h * W))
        in_maps.append({"fea": feab, "wsw": wsw, "w16": w16, "bias": bsb,
                        "id8": id8, "id16": id16})
    key = (h, fast_steps, bool(spmd_kwargs.pop("debug_plane", False)))
    if key not in _NC_CACHE:
        _NC_CACHE[key] = _build_nc(h, fast_steps, debug_plane=key[2])
    nc = _NC_CACHE[key]
    try:
        res = run_bass_kernel_spmd(
            nc, in_maps, core_ids=list(range(n)), trace=trace, **spmd_kwargs
        )
    except Exception:
        res = run_bass_kernel_spmd(
            nc, in_maps, core_ids=list(range(n)), trace=trace, **spmd_kwargs
        )
    outs = [res.results[bi]["out"].reshape(C, h, W) for bi in range(n)]
    return np.stack(outs, axis=0).astype(np.float32), res


def kernel(fea, weight, bias):
    out, _ = run(fea, weight, bias, trace=False)
    return out
